# revision 5
# baseline (speedup 1.0000x reference)
"""Multi-head attention (B=2, L=2048, D=1024, H=16, RoPE, softmax, out-proj)
on 8 Trainium2 NeuronCores — q-major pipelined rewrite.

Sharding: 2-way data parallel on batch x 4-way tensor parallel on heads
(core c: batch c//4, heads 4*(c%4)..+3), ReduceScatter after out-proj.

Per core, scalar-engine exp is the pipeline pacer (~145us of ACTIVATE):
  - attention runs q-major over 4 chunks of 512 q; per k-tile: S^T matmuls
    (2 heads into one [128,1024] PSUM tile), exp, PV trailing one k-tile
    (65-col matmuls accumulating [128,65] chains per (head, q-tile))
  - chunk 0's k-loop also interleaves the whole input pipeline: per 4 k-tiles
    it emits that 512-col chunk's QK-proj chains, V-proj, rope (split across
    DVE and gpsimd), and the qt/ktz scatter DMAs
  - chunk-finish work (normalize via ones-column reciprocal, PE transpose,
    out-proj, partial DMA, ReduceScatter, output DMA) is drained two items
    per k-iteration inside the NEXT chunk so the PE queue never serializes
Host reassembles [2, 2048, 1024] from the 8 per-core [256, 2048] outputs.
"""

import numpy as np
import ml_dtypes
from contextlib import ExitStack

import concourse.bass as bass
import concourse.tile as tile
from concourse import bacc, mybir
from concourse.bass_utils import run_bass_kernel_spmd
from concourse.masks import make_identity

BF16 = mybir.dt.bfloat16
F32 = mybir.dt.float32

B, L, D = 2, 2048, 1024
H_TOT, H = 16, 4          # total heads, heads per core
HD, HF = 64, 32           # head dim, rope freqs
DL = H * HD               # local head dims per core = 256
P = 128
KT = L // P               # 16 k-tiles
DK = D // P               # 8 contraction tiles over model dim
QC = 512                  # q chunk
NQC = L // QC             # 4 chunks
ROPE_BASE = 10000.0
GROUPS = [[0, 1, 2, 3], [4, 5, 6, 7]]

_CACHED_NC = None


def _build_program():
    nc = bacc.Bacc("TRN2", target_bir_lowering=False, debug=False, num_devices=8)

    xT_ext = nc.dram_tensor("xT", [DK, P, NQC, QC], BF16, kind="ExternalInput")
    wqk_ext = nc.dram_tensor("wqkT", [DK, P, 4 * P], BF16, kind="ExternalInput")
    wv_ext = nc.dram_tensor("wvT", [P, DK, DL], BF16, kind="ExternalInput")
    wo_ext = nc.dram_tensor("woT", [P, 2, D], BF16, kind="ExternalInput")
    cos_ext = nc.dram_tensor("cosF", [P, L], F32, kind="ExternalInput")
    sin_ext = nc.dram_tensor("sinF", [P, L], F32, kind="ExternalInput")
    out_ext = nc.dram_tensor("out", [NQC, DL, QC], BF16, kind="ExternalOutput")

    partials = [nc.dram_tensor(f"partialT{c}", [D, QC], BF16) for c in range(NQC)]
    scats = [nc.dram_tensor(f"scatT{c}", [DL, QC], BF16) for c in range(NQC)]

    Exp = mybir.ActivationFunctionType.Exp

    with tile.TileContext(nc) as tc:
        with ExitStack() as ctx:
            pers = ctx.enter_context(tc.tile_pool(name="pers", bufs=1))
            aps = ctx.enter_context(tc.tile_pool(name="aps", bufs=1, space="PSUM"))
            ptp = ctx.enter_context(tc.tile_pool(name="ptp", bufs=1))
            fin = ctx.enter_context(tc.tile_pool(name="fin", bufs=1))
            tmp = ctx.enter_context(tc.tile_pool(name="tmp", bufs=1))

            wqk = pers.tile([P, DK, 4 * P], BF16, tag="wqk")
            wv = pers.tile([P, DK, DL], BF16, tag="wv")
            wo = pers.tile([P, 2, D], BF16, tag="wo")
            cosf = pers.tile([P, L], F32, tag="cosf")
            sinf = pers.tile([P, L], F32, tag="sinf")
            xt = pers.tile([P, DK, NQC, QC], BF16, tag="xt")
            qt = pers.tile([P, 2, L], BF16, tag="qt")
            ktz = pers.tile([P, H, L], BF16, tag="ktz")
            v1 = pers.tile([P, KT, H * (HD + 1)], BF16, tag="v1")  # [V | 1]
            ident = pers.tile([P, P], F32, tag="ident")

            nc.vector.memset(ktz[:, 0:2, :], 0.0)
            nc.gpsimd.memset(ktz[:, 2:4, :], 0.0)

            # input DMAs: interleave wqk-dk with x-dk so chains start early
            for dk in range(DK):
                nc.sync.dma_start(out=wqk[:, dk, :], in_=wqk_ext[dk])
                nc.sync.dma_start(out=xt[:, dk, :, :], in_=xT_ext[dk])
                if dk == 1:
                    nc.sync.dma_start(out=cosf[:], in_=cos_ext[:])
                if dk == 3:
                    nc.sync.dma_start(out=sinf[:], in_=sin_ext[:])
            nc.sync.dma_start(out=wv[:], in_=wv_ext[:])
            nc.sync.dma_start(out=wo[:], in_=wo_ext[:])
            make_identity(nc, ident[:])

            def v_pair(k0):
                """V-proj for k-tiles k0, k0+1 into one sc tile; fill v1."""
                vp = aps.tile([P, QC], F32, tag="sc", bufs=1, name=f"vp{k0}")
                for j in range(2):
                    k = k0 + j
                    for dk in range(DK):
                        nc.tensor.matmul(
                            vp[:, j * DL:(j + 1) * DL],
                            xt[:, dk, k // 4, (k % 4) * P:(k % 4 + 1) * P],
                            wv[:, dk, :],
                            start=(j == 0 and dk == 0),
                            stop=(j == 1 and dk == DK - 1),
                            skip_group_check=True)
                for j in range(2):
                    k = k0 + j
                    src3 = vp[:, j * DL:(j + 1) * DL].rearrange(
                        "p (h d) -> p h d", h=H)
                    dst3 = v1[:, k, :].rearrange("p (h d) -> p h d", h=H)
                    nc.vector.tensor_copy(dst3[:, :, 0:HD], src3)
                    nc.gpsimd.memset(dst3[:, :, HD:HD + 1], 1.0)

            def _chains(c, half):
                stp = aps.tile([P, 2 * QC], F32, tag="st", bufs=2,
                               name=f"pq{c}_{half}")
                for mi, m in enumerate((2 * half, 2 * half + 1)):
                    for dk in range(DK):
                        nc.tensor.matmul(
                            stp[:, mi * QC:(mi + 1) * QC],
                            wqk[:, dk, m * P:(m + 1) * P],
                            xt[:, dk, c, :],
                            start=(dk == 0), stop=(dk == DK - 1))
                return stp

            def _rope_scatter(c, half, stp, qkr):
                # rope: r1 = x1*cos - x2*sin ; r2 = x1*sin + x2*cos
                cs = slice(c * QC, (c + 1) * QC)
                x1 = stp[:, 0:QC]
                x2 = stp[:, QC:2 * QC]
                base = 2 * half
                t1 = tmp.tile([P, QC], F32, tag="t1", bufs=2)
                t2 = tmp.tile([P, QC], F32, tag="t2", bufs=2)
                nc.vector.tensor_mul(t1[:], x1, cosf[:, cs])
                nc.vector.tensor_mul(t2[:], x2, sinf[:, cs])
                nc.gpsimd.tensor_sub(qkr[:, base, :], t1[:], t2[:])
                t3 = tmp.tile([P, QC], F32, tag="t1", bufs=2)
                t4 = tmp.tile([P, QC], F32, tag="t2", bufs=2)
                nc.vector.tensor_mul(t3[:], x1, sinf[:, cs])
                nc.vector.tensor_mul(t4[:], x2, cosf[:, cs])
                nc.gpsimd.tensor_add(qkr[:, base + 1, :], t3[:], t4[:])
                for h in range(H):
                    t, pb = h // 2, 64 * (h % 2)
                    hs = slice(32 * h, 32 * h + 32)
                    if half == 0:
                        nc.sync.dma_start(out=qt[pb:pb + 32, t, cs],
                                          in_=qkr[hs, 0, :])
                        nc.sync.dma_start(out=qt[pb + 32:pb + 64, t, cs],
                                          in_=qkr[hs, 1, :])
                    else:
                        nc.sync.dma_start(out=ktz[pb:pb + 32, h, cs],
                                          in_=qkr[hs, 2, :])
                        nc.sync.dma_start(out=ktz[pb + 32:pb + 64, h, cs],
                                          in_=qkr[hs, 3, :])

            def proj_slots(c):
                """Per-chunk proj work as 4 deferred emission slots."""
                state = {}

                def s0():
                    state['qkr'] = tmp.tile([P, 4, QC], BF16, tag="qkr",
                                            bufs=2, name=f"qkr{c}")
                    state['q'] = _chains(c, 0)

                def s1():
                    _rope_scatter(c, 0, state['q'], state['qkr'])

                def s2():
                    state['k'] = _chains(c, 1)

                def s3():
                    _rope_scatter(c, 1, state['k'], state['qkr'])
                return [s0, s1, s2, s3]

            def proj_chunk(c):
                for s in proj_slots(c):
                    s()

            # ---------------- attention, q-major ----------------
            tails = []

            def drain(n):
                for _ in range(n):
                    if tails:
                        tails.pop(0)()

            def _emit_s_exp(qc_, k, pts_):
                qcs_ = slice(qc_ * QC, (qc_ + 1) * QC)
                ks = slice(k * P, (k + 1) * P)
                for hp in range(2):
                    st = aps.tile([P, 2 * QC], F32, tag="st", bufs=2,
                                  name=f"st{qc_}_{k}_{hp}")
                    for hh in range(2):
                        h = 2 * hp + hh
                        nc.tensor.matmul(
                            st[:, hh * QC:(hh + 1) * QC],
                            ktz[:, h, ks], qt[:, hp, qcs_],
                            start=True, stop=True)
                    pt = ptp.tile([P, 2 * QC], BF16, tag="pt", bufs=7,
                                  name=f"pt{qc_}_{k}_{hp}")
                    nc.scalar.activation(pt[:], st[:], Exp)
                    pts_[(hp, k)] = pt

            def emit_pv(pvt, pts, k):
                # psum start=True zeroes a whole 2KB bank: only the bank's
                # first chain starts, only its last chain stops
                for hp in range(2):
                    pt = pts[(hp, k)]
                    for hh in range(2):
                        h = 2 * hp + hh
                        vs = slice(h * (HD + 1), (h + 1) * (HD + 1))
                        for qtile in range(4):
                            idx = 4 * h + qtile
                            first_in_bank = idx % 7 == 0
                            last_in_bank = idx in (6, 13, 15)
                            nc.tensor.matmul(
                                pvt[:, idx // 7, (idx % 7) * (HD + 1):
                                    (idx % 7 + 1) * (HD + 1)],
                                pt[:, hh * QC + qtile * P:hh * QC + (qtile + 1) * P],
                                v1[:, k, vs],
                                start=(k == 0 and first_in_bank),
                                stop=(k == KT - 1 and last_in_bank),
                                skip_group_check=True)

            pre = {}
            for qc in range(NQC):
                qcs = slice(qc * QC, (qc + 1) * QC)
                pvt = aps.tile([P, 3, QC], F32, tag="pv", bufs=1,
                               name=f"pv{qc}")
                o_nrm = fin.tile([P, 4, DL], F32, tag="onrm", bufs=2,
                                 name=f"onrm{qc}")
                pts = pre.pop(qc, {})
                for k in range(KT):
                    if qc == 0 and k == 0:
                        for c in range(NQC):
                            proj_chunk(c)
                    if qc == 0 and k % 2 == 0:
                        v_pair(k)
                    if (0, k) not in pts:
                        _emit_s_exp(qc, k, pts)
                    if k > 0:
                        emit_pv(pvt, pts, k - 1)
                        del pts[(0, k - 1)], pts[(1, k - 1)]
                    drain(8)
                if qc + 1 < NQC:
                    nxt = {}
                    _emit_s_exp(qc + 1, 0, nxt)
                    _emit_s_exp(qc + 1, 1, nxt)
                    pre[qc + 1] = nxt
                emit_pv(pvt, pts, KT - 1)

                for h in range(H):
                    for qtile in range(4):
                        idx = 4 * h + qtile
                        b, s = idx // 7, (idx % 7) * (HD + 1)
                        rec = fin.tile([P, 1], F32, tag="rec", bufs=8)
                        nc.vector.reciprocal(rec[:], pvt[:, b, s + HD:s + HD + 1])
                        nc.vector.tensor_scalar(
                            out=o_nrm[:, qtile, h * HD:(h + 1) * HD],
                            in0=pvt[:, b, s:s + HD],
                            scalar1=rec[:], scalar2=None,
                            op0=mybir.AluOpType.mult)

                tails.extend(_make_tail(nc, aps, fin, o_nrm, ident, wo,
                                        partials[qc], scats[qc], out_ext, qc))
                if qc == NQC - 1:
                    drain(len(tails))
            assert not tails
            with tc.tile_wait_until(1.0):
                for c in range(NQC):
                    nc.sync.dma_start(out=out_ext[c], in_=scats[c][:])

    nc.compile()
    return nc


def _make_tail(nc, aps, fin, o_nrm, ident, wo, partial, scat, out_ext, qc):
    """Closures for chunk-finish work, drained inside the next chunk."""
    onT = fin.tile([P, 2, QC], BF16, tag="onT", bufs=2, name=f"onT{qc}")
    items = []

    def tr(quad):
        def f():
            sc = aps.tile([P, QC], F32, tag="sc", bufs=1)
            for j, (qtile, dlh) in enumerate(quad):
                nc.tensor.matmul(
                    sc[:, j * P:(j + 1) * P],
                    o_nrm[:, qtile, dlh * P:(dlh + 1) * P], ident[:],
                    is_transpose=True,
                    start=(j == 0), stop=(j == len(quad) - 1),
                    skip_group_check=True)
            for j, (qtile, dlh) in enumerate(quad):
                nc.vector.tensor_copy(
                    onT[:, dlh, qtile * P:(qtile + 1) * P],
                    sc[:, j * P:(j + 1) * P])
        return f

    pieces = [(qtile, dlh) for qtile in range(4) for dlh in range(2)]
    items.append(tr(pieces[0:4]))
    items.append(tr(pieces[4:8]))

    def po(ot, last):
        def f():
            if last and ot % 2 == 1:
                pb = aps.tile([P, 2 * QC], F32, tag="st", bufs=2,
                              name=f"pot{ot}")
            else:
                pb = aps.tile([P, QC], F32, tag="sc", bufs=1,
                              name=f"pos{ot}")
            p = pb[:, 0:QC]
            for t in range(2):
                nc.tensor.matmul(p, wo[:, t, ot * P:(ot + 1) * P],
                                 onT[:, t, :], start=(t == 0), stop=(t == 1))
            so = fin.tile([P, QC], BF16, tag="so", bufs=8)
            nc.vector.tensor_copy(so[:], p)
            nc.sync.dma_start(out=partial[ot * P:(ot + 1) * P, :], in_=so[:])
        return f

    for ot in range(DK):
        items.append(po(ot, qc == NQC - 1))

    def finish():
        nc.gpsimd.collective_compute(
            "ReduceScatter", mybir.AluOpType.add,
            replica_groups=GROUPS,
            ins=[partial[:]], outs=[scat[:]])
    items.append(finish)
    return items


def _prep_inputs(x, W_qkv, W_out):
    """Host-side sharding / layout prep -> per-core input maps."""
    Wq, Wk, Wv = W_qkv[0:D], W_qkv[D:2 * D], W_qkv[2 * D:3 * D]
    inv = 1.0 / (ROPE_BASE ** (np.arange(0, HD, 2, dtype=np.float64) / HD))
    pos = np.arange(L, dtype=np.float64)
    ang = pos[:, None] * inv[None, :]                     # [L, 32]
    cosF = np.tile(np.cos(ang).T, (H, 1)).astype(np.float32)  # [128, L]
    sinF = np.tile(np.sin(ang).T, (H, 1)).astype(np.float32)

    scale = float(HD) ** -0.5
    in_maps = []
    for c in range(8):
        b, g = c // 4, c % 4
        rows_x1 = np.array([64 * (4 * g + h) + 2 * f
                            for h in range(H) for f in range(HF)])
        rows_x2 = rows_x1 + 1
        wqkT = np.concatenate([
            (scale * Wq[rows_x1]).T, (scale * Wq[rows_x2]).T,
            Wk[rows_x1].T, Wk[rows_x2].T], axis=1)        # [1024, 512]
        wvT = Wv[DL * g:DL * (g + 1)].T                   # [1024, 256]
        woT = W_out[:, DL * g:DL * (g + 1)].T             # [256, 1024]
        in_maps.append({
            "xT": np.ascontiguousarray(
                x[b].T.reshape(DK, P, NQC, QC)).astype(ml_dtypes.bfloat16),
            "wqkT": np.ascontiguousarray(
                wqkT.reshape(DK, P, 4 * P)).astype(ml_dtypes.bfloat16),
            "wvT": np.ascontiguousarray(
                wvT.reshape(DK, P, DL).transpose(1, 0, 2)).astype(ml_dtypes.bfloat16),
            "woT": np.ascontiguousarray(
                woT.reshape(2, P, D).transpose(1, 0, 2)).astype(ml_dtypes.bfloat16),
            "cosF": cosF, "sinF": sinF,
        })
    return in_maps


def _run(in_maps, trace=False):
    global _CACHED_NC
    if _CACHED_NC is None:
        _CACHED_NC = _build_program()
    kw = dict(trace=True) if trace else {}
    return run_bass_kernel_spmd(_CACHED_NC, in_maps, list(range(8)), **kw)


def kernel(x, W_qkv, W_out, _trace=False):
    x = np.asarray(x, dtype=np.float32)
    W_qkv = np.asarray(W_qkv, dtype=np.float32)
    W_out = np.asarray(W_out, dtype=np.float32)
    res = _run(_prep_inputs(x, W_qkv, W_out), trace=_trace)
    out = np.empty((B, L, D), dtype=np.float32)
    for b in range(B):
        outT = np.concatenate(
            [np.asarray(res.results[4 * b + j]["out"]).transpose(1, 0, 2)
             .reshape(DL, L) for j in range(4)], axis=0)
        out[b] = outT.T.astype(np.float32)
    if _trace:
        kernel.last_exec_time_ns = res.exec_time_ns
        kernel.last_trace = res.instructions_and_trace
    return out
